# revision 28
# baseline (speedup 1.0000x reference)
"""DRew-GCN forward on 8 Trainium2 NeuronCores — v2.

Node-partitioned (block-contiguous, graph-aligned). All GCN normalization
(dinv[src]*dinv[dst]/k per edge, dinv^2/k self terms) is precomputed on the
host and folded into one-hot scatter coefficients; one-hot Q tiles are
expanded on-chip from compressed (dstrel, coef) slabs with two chunked
broadcast DVE ops (bf16). Gather passes are slot-batched: hop k's edge rows
are fetched in ceil((6-k)/k) passes, each pulling several layer-stripes per
edge from a half-major wide table (descriptor count is independent of elem
width), and every covered (layer,hop) use is applied eagerly through its
conv weight into a per-layer xk accumulator (<=3 alive at once).
"""
import sys
import numpy as np

if '/opt/trn_rl_repo' not in sys.path:
    sys.path.insert(0, '/opt/trn_rl_repo')


class Cfg:
    def __init__(self, N=50000, G=500, IN_DIM=32, HID=128, OUT=10,
                 L=5, E_K=400000, NCORES=8):
        self.N, self.G, self.IN_DIM, self.HID, self.OUT = N, G, IN_DIM, HID, OUT
        self.L, self.E_K, self.NCORES = L, E_K, NCORES
        self.NPG = N // G
        assert N % G == 0
        self.GPC = -(-G // NCORES)              # graphs per core
        self.BLK = self.GPC * self.NPG          # real nodes per core (last less)
        self.NLOC = 128 * (-(-self.BLK // 128))
        self.W = self.NLOC // 128
        self.TBL = NCORES * self.NLOC
        self.PAIRS = self.TBL // 2
        assert self.PAIRS < 32768
        self.NSTRIPE = 4                        # wide-table stripes (x_0..x_3)
        self.GPAD = 512
        self.SMAX = 64
        self.SMAXR = self.GPC
        assert self.SMAXR <= self.SMAX
        self.QCH = 11                           # q-expansion chunk (pieces)
        self.NCONV = L * (L + 1) // 2

    def core_lo(self, c): return min(self.BLK * c, self.N)
    def core_hi(self, c): return min(self.BLK * (c + 1), self.N)

    def row_of(self, n):
        c = np.minimum(n // self.BLK, self.NCORES - 1)
        return self.NLOC * c + (n - self.BLK * c)


# Emission program: ('pass', hop, s0, src, j, nst[, (w_lo, w_hi)]) gathers
# hop's edges (optionally only dst windows [w_lo, w_hi)) reading stripes
# [s0, s0+nst) from tmp_j ('tmp'), the wide table, or a host-side
# pre-gathered stream ('host': x_0-sourced passes whose edge features are
# computable on the host and shipped as inputs -- no on-device dma_gather).
# ('AGstage', l): relu xk[l] -> hin_{l+1}, AllGather -> tmp_{l+1}.
# ('ilv', j): interleave tmp_j into wide stripe j. ('xkinit', l): seed
# xk[l]'s buffer with the folded per-layer bias (all passes then add).
# Pass order = Pool-engine queue order: collectives block later gathers,
# so hop-4 is split into window sub-passes that pad the AllGather waits.
PROG = [
    ('xkinit', 0),
    ('xkinit', 2),
    ('xkinit', 4),
    ('pass', 1, 0, 'host', 0, 1),
    ('AGstage', 0),
    ('xkinit', 1),
    ('pass', 5, 0, 'host', 4, 1, (0, 13)),
    ('ilv', 1),
    ('pass', 1, 1, 'tmp', 1, 1),
    ('pass', 2, 0, 'wide', None, 2),
    ('AGstage', 1),
    ('xkinit', 3),
    ('pass', 5, 0, 'host', 4, 1, (13, 26)),
    ('pass', 4, 0, 'wide', None, 2, (0, 25)),
    ('ilv', 2),
    ('pass', 3, 0, 'wide', None, 3),
    ('pass', 1, 2, 'tmp', 2, 1),
    ('AGstage', 2),
    ('pass', 5, 0, 'host', 4, 1, (26, 38)),
    ('pass', 4, 0, 'wide', None, 2, (25, None)),
    ('ilv', 3),
    ('pass', 2, 2, 'wide', None, 2),
    ('pass', 1, 3, 'tmp', 3, 1),
    ('AGstage', 3),
    ('pass', 5, 0, 'host', 4, 1, (38, None)),
    ('pass', 1, 4, 'tmp', 4, 1),
]
HOST_PASSES = [(p[1], p[2]) for p in PROG if p[0] == 'pass' and p[3] == 'host']
# xk accumulator buffer per layer (3 physical buffers)
XKMAP = {0: 0, 1: 0, 2: 1, 3: 0, 4: 2}
CH_OF_NST = {1: 40, 2: 20, 3: 13}


def ci_of(l, k):
    return l * (l + 1) // 2 + (k - 1)


def build_plan(cfg, k_edge_index, batch):
    NC, L, W = cfg.NCORES, cfg.L, cfg.W
    kei = np.asarray(k_edge_index)
    hops = []
    for k in range(1, L + 1):
        src = np.asarray(kei[0, (k - 1) * cfg.E_K: k * cfg.E_K], np.int64)
        dst = np.asarray(kei[1, (k - 1) * cfg.E_K: k * cfg.E_K], np.int64)
        deg = np.bincount(dst, minlength=cfg.N).astype(np.float64) + 1.0
        dinv = 1.0 / np.sqrt(deg)
        per_core = []
        T = np.zeros((W, 2), np.int64)
        for c in range(NC):
            lo, hi = cfg.core_lo(c), cfg.core_hi(c)
            m = (dst >= lo) & (dst < hi)
            es, ed = src[m], dst[m]
            erow = cfg.row_of(es)
            half = (erow & 1).astype(np.int64)
            pair = (erow >> 1).astype(np.int64)
            dl = ed - lo
            w = dl >> 7
            dr = dl & 127
            coefe = (dinv[es] * dinv[ed] / k).astype(np.float32)
            per_core.append((pair, half, w, dr, coefe, es))
            for h in (0, 1):
                cnt = np.bincount(w[half == h], minlength=W)
                T[:, h] = np.maximum(T[:, h], -(-cnt // 128))
        NT = [int(T[:, 0].sum()), int(T[:, 1].sum())]
        P = NT[0] + NT[1] + W
        # per-window bases
        TOFF = np.zeros((W, 2), np.int64)       # first stream-tile of window
        TOFF[1:, 0] = np.cumsum(T[:-1, 0])
        TOFF[1:, 1] = np.cumsum(T[:-1, 1])
        PB = np.zeros(W, np.int64)              # first piece of window
        PB[1:] = np.cumsum(T[:-1, 0] + T[:-1, 1] + 1)
        # piece index of each stream tile
        piece_of_tile = [np.zeros(max(NT[h], 1), np.int64) for h in (0, 1)]
        for w in range(W):
            for h in (0, 1):
                t0 = TOFF[w, h]
                pb = PB[w] + (T[w, 0] if h == 1 else 0)
                piece_of_tile[h][t0:t0 + T[w, h]] = pb + np.arange(T[w, h])
        idx_all = np.zeros((NC, 2, max(NT[0], 1) * 128), np.int16)
        dsr_all = np.full((NC, 128, P), 255.0, np.float32)
        coef_all = np.zeros((NC, 128, P), np.float32)
        if NT[1] != NT[0]:
            idx_all = np.zeros((NC, 2, max(NT[0], NT[1]) * 128), np.int16)
        src_at_pos = np.full((NC, 2, idx_all.shape[2]), -1, np.int64)
        for c in range(NC):
            pair, half, w, dr, coefe, esrc = per_core[c]
            lo, hi = cfg.core_lo(c), cfg.core_hi(c)
            for h in (0, 1):
                sel = half == h
                ww, pp, drr, cc = w[sel], pair[sel], dr[sel], coefe[sel]
                ee = esrc[sel]
                order = np.argsort(ww, kind='stable')
                ww, pp, drr, cc = ww[order], pp[order], drr[order], cc[order]
                ee = ee[order]
                cnts = np.bincount(ww, minlength=W)
                grp0 = np.concatenate([[0], np.cumsum(cnts)[:-1]])
                pos = TOFF[ww, h] * 128 + (np.arange(len(ww)) -
                                           np.repeat(grp0, cnts))
                idx_all[c, h, pos] = pp.astype(np.int16)
                src_at_pos[c, h, pos] = ee
                tg = pos >> 7
                part = pos & 127
                pc = piece_of_tile[h][tg]
                dsr_all[c, part, pc] = drr
                coef_all[c, part, pc] = cc
            # self pieces
            for w_ in range(W):
                pc = PB[w_] + T[w_, 0] + T[w_, 1]
                node = lo + w_ * 128 + np.arange(128)
                real = node < hi
                dsr_all[c, :, pc] = np.arange(128)
                cs = np.zeros(128, np.float32)
                cs[real] = (dinv[node[real]] ** 2 / k).astype(np.float32)
                coef_all[c, :, pc] = cs
                dsr_all[c, ~real, pc] = 255.0
        hops.append(dict(T=T, NT=NT, P=P, TOFF=TOFF, PB=PB,
                         idx=idx_all, dsr=dsr_all, coef=coef_all,
                         src=src_at_pos))
    # pooling slice->graph matrix
    Pm = np.zeros((NC, cfg.SMAX, cfg.GPAD), np.float32)
    for c in range(NC):
        lo, hi = cfg.core_lo(c), cfg.core_hi(c)
        gbase = lo // cfg.NPG
        for s in range((hi - lo) // cfg.NPG):
            Pm[c, s, gbase + s] = 1.0
    b = np.asarray(batch, np.int64)
    cnt = np.bincount(b, minlength=cfg.G)
    assert (cnt == cfg.NPG).all() and (np.sort(b) == b).all()
    key = tuple((h['NT'][0], h['NT'][1], h['P']) for h in hops)
    return dict(hops=hops, P=Pm, key=key)


def build_bass(cfg, plan):
    import concourse.bacc as bacc
    import concourse.mybir as mybir
    from concourse.tile import TileContext
    from concourse.library_config import mlp as mlp_lib

    f32, bf16, i16 = mybir.dt.float32, mybir.dt.bfloat16, mybir.dt.int16
    Alu = mybir.AluOpType
    Act = mybir.ActivationFunctionType
    AX = mybir.AxisListType.X
    NC, L, W = cfg.NCORES, cfg.L, cfg.W
    HID, GPAD, QCH = cfg.HID, cfg.GPAD, cfg.QCH
    RG = [list(range(NC))]

    nc = bacc.Bacc("TRN2", num_devices=NC)

    xT = nc.dram_tensor("xT", [cfg.IN_DIM, cfg.NLOC], f32, kind="ExternalInput")
    host_hops = {k for k, _ in HOST_PASSES}
    dev_hops = {op[1] for op in PROG if op[0] == 'pass' and op[3] != 'host'}
    idx_d, q_d, hostG_d = {}, {}, {}
    for k in range(1, L + 1):
        hp = plan['hops'][k - 1]
        ntmax = max(hp['NT'][0], hp['NT'][1], 1)
        if k in dev_hops:
            idx_d[k] = nc.dram_tensor(f"idx{k}", [2, 128, ntmax * 8], i16,
                                      kind="ExternalInput")
        q_d[k] = nc.dram_tensor(f"q{k}", [128, hp['P'] * 128], bf16,
                                kind="ExternalInput")
        if k in host_hops:
            hostG_d[k] = nc.dram_tensor(
                f"hg{k}", [128, (hp['NT'][0] + hp['NT'][1]) * 128], bf16,
                kind="ExternalInput")
    ident_d = nc.dram_tensor("ident", [128, 128], f32, kind="ExternalInput")
    P_d = nc.dram_tensor("P", [cfg.SMAX, GPAD], f32, kind="ExternalInput")
    embWT_d = nc.dram_tensor("embWT", [cfg.IN_DIM, HID], f32, kind="ExternalInput")
    embB_d = nc.dram_tensor("embB", [1, HID], f32, kind="ExternalInput")
    convWT_d = nc.dram_tensor("convWT", [cfg.NCONV, HID, HID], bf16,
                              kind="ExternalInput")
    biasL_d = nc.dram_tensor("biasL", [1, L * HID], bf16, kind="ExternalInput")
    r1WT_d = nc.dram_tensor("r1WT", [3 * HID, 192], f32, kind="ExternalInput")
    r1B_d = nc.dram_tensor("r1B", [192, 1], f32, kind="ExternalInput")
    r2WT_d = nc.dram_tensor("r2WT", [192, cfg.OUT], f32, kind="ExternalInput")
    r2B_d = nc.dram_tensor("r2B", [cfg.OUT, 1], f32, kind="ExternalInput")
    y_d = nc.dram_tensor("y", [cfg.OUT, GPAD], f32, kind="ExternalOutput")

    tmp = [nc.dram_tensor(f"tmp{j}", [cfg.TBL, HID], bf16, kind="Internal",
                          addr_space="Shared") for j in range(L)]
    wide = nc.dram_tensor("wide", [cfg.PAIRS, 2 * cfg.NSTRIPE * HID], bf16,
                          kind="Internal")
    hin = [nc.dram_tensor(f"hin{j}", [cfg.NLOC, HID], bf16, kind="Internal")
           for j in range(L)]
    ps_in = [nc.dram_tensor(f"pool_in{i}", [128, GPAD], f32, kind="Internal")
             for i in range(2)]
    ps_out = [nc.dram_tensor(f"pool_out{i}", [128, GPAD], f32, kind="Internal",
                             addr_space="Shared") for i in range(2)]

    # coverage check: every layer's xk gets seeded and every conv lands once
    seen = set()
    for op in PROG:
        if op[0] != 'pass':
            continue
        k, s0, nst = op[1], op[2], op[5]
        for jj in range(nst):
            seen.add(s0 + jj + k - 1)
    assert seen == {0, 1, 2, 3, 4}

    with TileContext(nc) as tc:
        nc.gpsimd.load_library(mlp_lib)
        with tc.tile_pool(name="const", bufs=1) as constp, \
             tc.tile_pool(name="persist", bufs=1) as pers, \
             tc.tile_pool(name="io", bufs=2) as iop, \
             tc.tile_pool(name="gp", bufs=2) as gp, \
             tc.tile_pool(name="qp", bufs=3) as qp, \
             tc.tile_pool(name="atp", bufs=3) as atp, \
             tc.tile_pool(name="selfp", bufs=3) as selfp, \
             tc.tile_pool(name="agg", bufs=4, space="PSUM") as aggp, \
             tc.tile_pool(name="outp", bufs=2, space="PSUM") as outp, \
             tc.tile_pool(name="smallps", bufs=2, space="PSUM") as smallp:

            ident = constp.tile([128, 128], f32)
            nc.sync.dma_start(ident[:], ident_d[:])
            ones_row = constp.tile([1, 128], f32)
            nc.vector.memset(ones_row[:], 1.0)
            ones_bf = constp.tile([1, 128], bf16)
            nc.vector.memset(ones_bf[:], 1.0)

            xkbuf = [pers.tile([128, W, HID], f32, tag=f"xk{i}",
                               name=f"xk{i}") for i in range(3)]
            h5T = xkbuf[0]      # dead by phase E; reuse as transpose staging

            blrow = constp.tile([1, L * HID], bf16)
            nc.sync.dma_start(blrow[:], biasL_d[:])
            biasbc = []
            for l in range(L):
                bps = smallp.tile([128, 128], f32, tag="smallt",
                                  name=f"bps{l}")
                nc.tensor.matmul(bps[:], ones_bf[:],
                                 blrow[0:1, l * HID:(l + 1) * HID])
                bb = constp.tile([128, 128], f32, name=f"biasbc{l}")
                nc.vector.tensor_copy(bb[:], bps[:])
                biasbc.append(bb)

            def emit_xkinit(l):
                nc.vector.tensor_copy(
                    xkbuf[XKMAP[l]][:, :, :],
                    biasbc[l][:, None, :].broadcast_to([128, W, HID]))

            # ---- Phase A: h0 = x @ embW^T + emb_b -> hin0 -> AG tmp0
            embWT = constp.tile([cfg.IN_DIM, HID], f32)
            nc.sync.dma_start(embWT[:], embWT_d[:])
            embB = constp.tile([1, HID], f32)
            nc.sync.dma_start(embB[:], embB_d[:])
            bias_ps = smallp.tile([128, 128], f32, tag="smallt")
            nc.tensor.matmul(bias_ps[:], ones_row[:], embB[:])
            embB_bc = constp.tile([128, 128], f32)
            nc.vector.tensor_copy(embB_bc[:], bias_ps[:])
            h0bf = pers.tile([128, W, HID], bf16, tag="hstage")
            for w in range(W):
                xtw = iop.tile([cfg.IN_DIM, 128], f32, tag="xtw")
                nc.sync.dma_start(xtw[:], xT[:, w * 128:(w + 1) * 128])
                hps = outp.tile([128, HID], f32, tag="ops")
                nc.tensor.matmul(hps[:], xtw[:], embWT[:])
                nc.vector.tensor_tensor(h0bf[:, w, :], hps[:], embB_bc[:], Alu.add)
            hview0 = hin[0].rearrange("(w j) f -> j w f", j=128)
            nc.sync.dma_start(hview0[:, :, :], h0bf[:])
            nc.gpsimd.collective_compute("AllGather", Alu.bypass,
                                         replica_groups=RG,
                                         ins=[hin[0][:]], outs=[tmp[0][:]])
            # interleave stripe 0 into wide
            wide_v = wide.rearrange("r (h s f) -> r h s f", h=2, s=cfg.NSTRIPE)
            tmp0_v = tmp[0].rearrange("(r two) f -> r two f", two=2)
            nc.sync.dma_start(wide_v[:, :, 0, :], tmp0_v[:])

            def emit_ilv(j):
                tv = tmp[j].rearrange("(r two) f -> r two f", two=2)
                nc.sync.dma_start(wide_v[:, :, j, :], tv[:])

            def emit_pass(k, s0, srck, tj, nst, wr=None):
                hp = plan['hops'][k - 1]
                T, NT, P, TOFF, PB = (hp['T'], hp['NT'], hp['P'],
                                      hp['TOFF'], hp['PB'])
                w_lo, w_hi = (0, W) if wr is None else wr
                if w_hi is None:
                    w_hi = W
                tbase = [int(TOFF[w_lo, 0]), int(TOFF[w_lo, 1])]
                tend = [int(TOFF[w_hi, h]) if w_hi < W else int(NT[h])
                        for h in (0, 1)]
                CH = CH_OF_NST[nst]
                uses = [(s0 + jj + k - 1, jj) for jj in range(nst)]
                wts = []
                for jj, (lp, _) in enumerate(uses):
                    wt = iop.tile([128, HID], bf16, tag=f"wt{jj}",
                                  name=f"wt{jj}")
                    nc.sync.dma_start(wt[:], convWT_d[ci_of(lp, k), :, :])
                    wts.append(wt)
                # gather source APs
                if srck == 'tmp':
                    tv = tmp[tj].rearrange("(r two) f -> r (two f)", two=2)
                    srcs = [tv[:, 0:HID], tv[:, HID:2 * HID]]
                    estep = 2 * HID
                elif srck == 'wide':
                    hw = cfg.NSTRIPE * HID
                    srcs = [wide[:, s0 * HID:(s0 + nst) * HID],
                            wide[:, hw + s0 * HID:hw + (s0 + nst) * HID]]
                    estep = 2 * hw

                if srck != 'host':
                    idx_sb = []
                    for h in (0, 1):
                        nth = max(NT[h], 1)
                        it = iop.tile([128, nth * 8], i16, tag=f"islab{h}",
                                      name=f"islab{h}")
                        nc.sync.dma_start(it[:], idx_d[k][h, :, 0:nth * 8])
                        idx_sb.append(it)

                gstate = [dict(c=-1, buf=None), dict(c=-1, buf=None)]

                def gtile(h, t):
                    st = gstate[h]
                    c = (t - tbase[h]) // CH
                    if c != st['c']:
                        t0 = tbase[h] + c * CH
                        n_t = min(CH, tend[h] - t0)
                        buf = gp.tile([128, CH, nst * HID], bf16, tag=f"g{h}")
                        if srck == 'host':
                            hoff = 0 if h == 0 else NT[0]
                            nc.scalar.dma_start(
                                buf[:, 0:n_t, :].rearrange("p t f -> p (t f)"),
                                hostG_d[k][:, (hoff + t0) * HID:
                                           (hoff + t0 + n_t) * HID])
                        else:
                            nc.gpsimd.dma_gather(buf[:, 0:n_t, :], srcs[h],
                                                 idx_sb[h][:, t0 * 8:
                                                           (t0 + n_t) * 8],
                                                 n_t * 128, n_t * 128,
                                                 nst * HID, elem_step=estep,
                                                 single_packet=False)
                        st['c'], st['buf'], st['t0'] = c, buf, t0
                    return st['buf'][:, t - st['t0'], :]

                qstate = dict(c=-1, qv=None)

                def qpiece(p):
                    st = qstate
                    c = p // QCH
                    if c != st['c']:
                        n_p = min(QCH, P - c * QCH)
                        qv = qp.tile([128, QCH, 128], bf16, tag="qv", bufs=6)
                        nc.scalar.dma_start(
                            qv[:, 0:n_p, :].rearrange("p t f -> p (t f)"),
                            q_d[k][:, c * QCH * 128:(c * QCH + n_p) * 128])
                        st['c'], st['qv'] = c, qv
                    return st['qv'][:, p - st['c'] * QCH, :]

                for w in range(w_lo, w_hi):
                    npieces = int(T[w, 0] + T[w, 1] + 1)
                    selfbs = []
                    for jj in range(nst):
                        sb = selfp.tile([128, HID], bf16, tag=f"sf{jj}")
                        nc.sync.dma_start(
                            sb[:], hin[s0 + jj][w * 128:(w + 1) * 128, :])
                        selfbs.append(sb)
                    aggs = [aggp.tile([128, 128], f32, tag="agg",
                                      name=f"agg{jj}") for jj in range(nst)]
                    for pi in range(npieces):
                        q = qpiece(int(PB[w]) + pi)
                        first, last = pi == 0, pi == npieces - 1
                        if pi < T[w, 0]:
                            base = gtile(0, int(TOFF[w, 0]) + pi)
                            lhs = [base[:, jj * HID:(jj + 1) * HID]
                                   for jj in range(nst)]
                        elif pi < T[w, 0] + T[w, 1]:
                            t1 = pi - int(T[w, 0])
                            base = gtile(1, int(TOFF[w, 1]) + t1)
                            lhs = [base[:, jj * HID:(jj + 1) * HID]
                                   for jj in range(nst)]
                        else:
                            lhs = [selfbs[jj][:] for jj in range(nst)]
                        for jj in range(nst):
                            nc.tensor.matmul(aggs[jj][:], lhs[jj], q,
                                             start=first, stop=last)
                    for jj, (lp, _) in enumerate(uses):
                        at = atp.tile([128, 128], bf16, tag="at")
                        nc.scalar.copy(at[:], aggs[jj][:])
                        ops = outp.tile([128, 128], f32, tag="ops")
                        nc.tensor.matmul(ops[:], at[:], wts[jj][:],
                                         start=True, stop=True)
                        xkw = xkbuf[XKMAP[lp]][:, w, :]
                        nc.vector.tensor_tensor(xkw, xkw, ops[:], Alu.add)

            def emit_agstage(l):
                hbf = pers.tile([128, W, HID], bf16, tag="hstage")
                nc.scalar.activation(hbf[:], xkbuf[XKMAP[l]][:], Act.Relu)
                hv = hin[l + 1].rearrange("(w j) f -> j w f", j=128)
                nc.sync.dma_start(hv[:, :, :], hbf[:])
                nc.gpsimd.collective_compute(
                    "AllGather", Alu.bypass, replica_groups=RG,
                    ins=[hin[l + 1][:]], outs=[tmp[l + 1][:]])

            for op in PROG:
                if op[0] == 'pass':
                    _, k, s0, srck, tj, nst, *rest = op
                    emit_pass(k, s0, srck, tj, nst,
                              rest[0] if rest else None)
                elif op[0] == 'AGstage':
                    emit_agstage(op[1])
                elif op[0] == 'ilv':
                    emit_ilv(op[1])
                elif op[0] == 'xkinit':
                    emit_xkinit(op[1])

            # ---- final relu + pooling + MLP (per-window so Tile overlaps
            # the tail with the last gather pass)
            xk = xkbuf[XKMAP[L - 1]]
            for w in range(W):
                nc.scalar.activation(xk[:, w, :], xk[:, w, :], Act.Relu)
                tp = outp.tile([128, 128], f32, tag="ops")
                nc.tensor.transpose(tp[:], xk[:, w, :], ident[:])
                nc.vector.tensor_copy(h5T[:, w, :], tp[:])
            h5flat = h5T[:].rearrange("p w j -> p (w j)")
            ssum_l = iop.tile([128, cfg.SMAX], f32, tag="ssum_l", bufs=1)
            smax_l = iop.tile([128, cfg.SMAX], f32, tag="smax_l", bufs=1)
            nc.vector.memset(ssum_l[:], 0.0)
            nc.vector.memset(smax_l[:], 0.0)
            for s in range(cfg.SMAXR):
                sl = h5flat[:, s * cfg.NPG:(s + 1) * cfg.NPG]
                nc.vector.tensor_reduce(ssum_l[:, s:s + 1], sl, axis=AX,
                                        op=Alu.add)
                nc.vector.tensor_reduce(smax_l[:, s:s + 1], sl, axis=AX,
                                        op=Alu.max)
            Ps = constp.tile([cfg.SMAX, GPAD], f32)
            nc.sync.dma_start(Ps[:], P_d[:])
            ssum_t = iop.tile([cfg.SMAX, 128], f32, tag="ssum_t", bufs=1)
            smax_t = iop.tile([cfg.SMAX, 128], f32, tag="smax_t", bufs=1)
            for tsb, tsl in ((ssum_t, ssum_l), (smax_t, smax_l)):
                tps2 = smallp.tile([cfg.SMAX, 128], f32, tag="smallt",
                                   name="tps2")
                nc.tensor.transpose(tps2[:], tsl[:], ident[:])
                nc.vector.tensor_copy(tsb[:], tps2[:])
            for i, tsrc in enumerate((ssum_t, smax_t)):
                pps = outp.tile([128, GPAD], f32, tag="ops")
                nc.tensor.matmul(pps[:], tsrc[:], Ps[:])
                psb = iop.tile([128, GPAD], f32, tag=f"psb{i}", bufs=1)
                nc.vector.tensor_copy(psb[:], pps[:])
                nc.sync.dma_start(ps_in[i][:], psb[:])
            nc.gpsimd.collective_compute("AllReduce", Alu.add, replica_groups=RG,
                                         ins=[ps_in[0][:]], outs=[ps_out[0][:]])
            nc.gpsimd.collective_compute("AllReduce", Alu.max, replica_groups=RG,
                                         ins=[ps_in[1][:]], outs=[ps_out[1][:]])
            ssumR = pers.tile([128, GPAD], f32)
            nc.sync.dma_start(ssumR[:], ps_out[0][:])
            smaxR = pers.tile([128, GPAD], f32)
            nc.sync.dma_start(smaxR[:], ps_out[1][:])

            r1WTs = [constp.tile([HID, 192], f32, name=f"r1w{j}")
                     for j in range(3)]
            for j in range(3):
                nc.sync.dma_start(r1WTs[j][:], r1WT_d[j * HID:(j + 1) * HID, :])
            r1Bs = [constp.tile([128, 1], f32, name="r1b0"),
                    constp.tile([64, 1], f32, name="r1b1")]
            nc.sync.dma_start(r1Bs[0][:], r1B_d[0:128, :])
            nc.sync.dma_start(r1Bs[1][:], r1B_d[128:192, :])
            r2WTs = [constp.tile([128, cfg.OUT], f32, name="r2w0"),
                     constp.tile([64, cfg.OUT], f32, name="r2w1")]
            nc.sync.dma_start(r2WTs[0][:], r2WT_d[0:128, :])
            nc.sync.dma_start(r2WTs[1][:], r2WT_d[128:192, :])
            r2Bs = constp.tile([cfg.OUT, 1], f32)
            nc.sync.dma_start(r2Bs[:], r2B_d[:])
            chunks = (ssumR, smaxR, ssumR)   # mean folded into r1WT scale
            hidT = []
            for mi, (m0, m1) in enumerate(((0, 128), (128, 192))):
                hps2 = outp.tile([m1 - m0, GPAD], f32, tag="ops", name="hps2")
                for j in range(3):
                    nc.tensor.matmul(hps2[:], r1WTs[j][:, m0:m1],
                                     chunks[j][:], start=(j == 0), stop=(j == 2))
                hsb = iop.tile([m1 - m0, GPAD], f32, tag=f"hsb{m0}", bufs=1,
                               name=f"hsb{m0}")
                nc.scalar.activation(hsb[:], hps2[:], Act.Lrelu,
                                     bias=r1Bs[mi][:, 0:1], alpha=0.01)
                hidT.append(hsb)
            yps = outp.tile([cfg.OUT, GPAD], f32, tag="ops")
            nc.tensor.matmul(yps[:], r2WTs[0][:], hidT[0][:],
                             start=True, stop=False)
            nc.tensor.matmul(yps[:], r2WTs[1][:], hidT[1][:],
                             start=False, stop=True)
            ysb = iop.tile([cfg.OUT, GPAD], f32, tag="ysb", bufs=1)
            nc.scalar.activation(ysb[:], yps[:], Act.Identity,
                                 bias=r2Bs[:, 0:1])
            nc.sync.dma_start(y_d[:], ysb[:])

    nc.compile()
    return nc


def _wrap_idx(arr):
    w16 = arr.reshape(-1, 16).T
    return np.tile(w16, (8, 1)).copy()


def make_inmaps(cfg, plan, inputs):
    import ml_dtypes
    bf = ml_dtypes.bfloat16
    x = np.asarray(inputs['x'], np.float32)
    emb_W = np.asarray(inputs['emb_W'], np.float32)
    emb_b = np.asarray(inputs['emb_b'], np.float32)
    x0bf = (x @ emb_W.T + emb_b).astype(bf)
    host_hops = {k for k, _ in HOST_PASSES}
    conv_b = np.asarray(inputs['conv_b'], np.float32)
    biasL = np.zeros((cfg.L, cfg.HID), np.float32)
    for l in range(cfg.L):
        for k in range(1, l + 2):
            biasL[l] += conv_b[ci_of(l, k)] / k
    shared = dict(
        ident=np.eye(128, dtype=np.float32),
        embWT=np.ascontiguousarray(np.asarray(inputs['emb_W'], np.float32).T),
        embB=np.asarray(inputs['emb_b'], np.float32)[None, :].copy(),
        convWT=np.ascontiguousarray(
            np.asarray(inputs['conv_W'], np.float32).transpose(0, 2, 1)
        ).astype(bf),
        biasL=biasL.astype(bf).reshape(1, -1),
        r1WT=np.ascontiguousarray(
            np.asarray(inputs['r1_W'], np.float32).T
            * np.concatenate([np.ones(256, np.float32),
                              np.full(128, 0.01, np.float32)])[:, None]),
        r1B=np.asarray(inputs['r1_b'], np.float32)[:, None].copy(),
        r2WT=np.ascontiguousarray(np.asarray(inputs['r2_W'], np.float32).T),
        r2B=np.asarray(inputs['r2_b'], np.float32)[:, None].copy(),
    )
    in_maps = []
    for c in range(cfg.NCORES):
        m = dict(shared)
        lo, hi = cfg.core_lo(c), cfg.core_hi(c)
        xs = np.zeros((cfg.NLOC, cfg.IN_DIM), np.float32)
        xs[:hi - lo] = x[lo:hi]
        m['xT'] = np.ascontiguousarray(xs.T)
        for k in range(1, cfg.L + 1):
            hp = plan['hops'][k - 1]
            ntmax = max(hp['NT'][0], hp['NT'][1], 1)
            if k not in host_hops or k == 1:
                wrapped = np.zeros((2, 128, ntmax * 8), np.int16)
                for h in (0, 1):
                    nth = max(hp['NT'][h], 1)
                    wr = _wrap_idx(hp['idx'][c, h, :nth * 128])
                    wrapped[h, :, :nth * 8] = wr
                m[f'idx{k}'] = wrapped
            oneh = (np.arange(128, dtype=np.float32)[None, None, :] ==
                    hp['dsr'][c][:, :, None])
            qfull = (oneh * hp['coef'][c][:, :, None]).astype(bf)
            m[f'q{k}'] = np.ascontiguousarray(qfull.reshape(128, -1))
            if k in host_hops:
                NT0, NT1 = int(hp['NT'][0]), int(hp['NT'][1])
                full = np.zeros((128, NT0 + NT1, 128), bf)
                for h, base, nth in ((0, 0, NT0), (1, NT0, NT1)):
                    s = hp['src'][c, h, :nth * 128]
                    arr = x0bf[np.clip(s, 0, None)]
                    arr[s < 0] = 0
                    full[:, base:base + nth, :] = (
                        arr.reshape(nth, 128, 128).transpose(1, 0, 2))
                m[f'hg{k}'] = np.ascontiguousarray(full.reshape(128, -1))
        m['P'] = plan['P'][c]
        in_maps.append(m)
    return in_maps


_CACHE = {}


def kernel(**inputs):
    from concourse.bass_utils import run_bass_kernel_spmd
    cfg = Cfg()
    plan = build_plan(cfg, np.asarray(inputs['k_edge_index']),
                      np.asarray(inputs['batch']))
    if plan['key'] not in _CACHE:
        _CACHE[plan['key']] = build_bass(cfg, plan)
    nc = _CACHE[plan['key']]
    in_maps = make_inmaps(cfg, plan, inputs)
    res = run_bass_kernel_spmd(nc, in_maps, core_ids=list(range(cfg.NCORES)))
    out = np.asarray(res.results[0]['y'], np.float32)
    return np.ascontiguousarray(out[:, :cfg.G].T)



# revision 29
# speedup vs baseline: 1.0047x; 1.0047x over previous
"""DRew-GCN forward on 8 Trainium2 NeuronCores — v2.

Node-partitioned (block-contiguous, graph-aligned). All GCN normalization
(dinv[src]*dinv[dst]/k per edge, dinv^2/k self terms) is precomputed on the
host and folded into one-hot scatter coefficients; one-hot Q tiles are
expanded on-chip from compressed (dstrel, coef) slabs with two chunked
broadcast DVE ops (bf16). Gather passes are slot-batched: hop k's edge rows
are fetched in ceil((6-k)/k) passes, each pulling several layer-stripes per
edge from a half-major wide table (descriptor count is independent of elem
width), and every covered (layer,hop) use is applied eagerly through its
conv weight into a per-layer xk accumulator (<=3 alive at once).
"""
import sys
import numpy as np

if '/opt/trn_rl_repo' not in sys.path:
    sys.path.insert(0, '/opt/trn_rl_repo')


class Cfg:
    def __init__(self, N=50000, G=500, IN_DIM=32, HID=128, OUT=10,
                 L=5, E_K=400000, NCORES=8):
        self.N, self.G, self.IN_DIM, self.HID, self.OUT = N, G, IN_DIM, HID, OUT
        self.L, self.E_K, self.NCORES = L, E_K, NCORES
        self.NPG = N // G
        assert N % G == 0
        self.GPC = -(-G // NCORES)              # graphs per core
        self.BLK = self.GPC * self.NPG          # real nodes per core (last less)
        self.NLOC = 128 * (-(-self.BLK // 128))
        self.W = self.NLOC // 128
        self.TBL = NCORES * self.NLOC
        self.PAIRS = self.TBL // 2
        assert self.PAIRS < 32768
        self.NSTRIPE = 4                        # wide-table stripes (x_0..x_3)
        self.GPAD = 512
        self.SMAX = 64
        self.SMAXR = self.GPC
        assert self.SMAXR <= self.SMAX
        self.QCH = 11                           # q-expansion chunk (pieces)
        self.NCONV = L * (L + 1) // 2

    def core_lo(self, c): return min(self.BLK * c, self.N)
    def core_hi(self, c): return min(self.BLK * (c + 1), self.N)

    def row_of(self, n):
        c = np.minimum(n // self.BLK, self.NCORES - 1)
        return self.NLOC * c + (n - self.BLK * c)


# Emission program: ('pass', hop, s0, src, j, nst[, (w_lo, w_hi)]) gathers
# hop's edges (optionally only dst windows [w_lo, w_hi)) reading stripes
# [s0, s0+nst) from tmp_j ('tmp'), the wide table, or a host-side
# pre-gathered stream ('host': x_0-sourced passes whose edge features are
# computable on the host and shipped as inputs -- no on-device dma_gather).
# ('AGstage', l): relu xk[l] -> hin_{l+1}, AllGather -> tmp_{l+1}.
# ('ilv', j): interleave tmp_j into wide stripe j. ('xkinit', l): seed
# xk[l]'s buffer with the folded per-layer bias (all passes then add).
# Pass order = Pool-engine queue order: collectives block later gathers,
# so hop-4 is split into window sub-passes that pad the AllGather waits.
PROG = [
    ('xkinit', 0),
    ('xkinit', 2),
    ('xkinit', 4),
    ('pass', 1, 0, 'host', 0, 1),
    ('AGstage', 0),
    ('xkinit', 1),
    ('pass', 5, 0, 'host', 4, 1, (0, 13)),
    ('ilv', 1),
    ('pass', 1, 1, 'tmp', 1, 1),
    ('pass', 2, 0, 'wide', None, 2),
    ('AGstage', 1),
    ('xkinit', 3),
    ('pass', 5, 0, 'host', 4, 1, (13, 26)),
    ('pass', 4, 0, 'wide', None, 2, (0, 25)),
    ('ilv', 2),
    ('pass', 3, 0, 'wide', None, 3),
    ('pass', 1, 2, 'tmp', 2, 1),
    ('AGstage', 2),
    ('pass', 5, 0, 'host', 4, 1, (26, 38)),
    ('pass', 4, 0, 'wide', None, 2, (25, None)),
    ('ilv', 3),
    ('pass', 2, 2, 'wide', None, 2),
    ('pass', 1, 3, 'tmp', 3, 1),
    ('AGstage', 3),
    ('pass', 5, 0, 'host', 4, 1, (38, None)),
    ('pass', 1, 4, 'tmp', 4, 1),
]
HOST_PASSES = [(p[1], p[2]) for p in PROG if p[0] == 'pass' and p[3] == 'host']
# xk accumulator buffer per layer (3 physical buffers)
XKMAP = {0: 0, 1: 0, 2: 1, 3: 0, 4: 2}
CH_OF_NST = {1: 40, 2: 20, 3: 13}


def ci_of(l, k):
    return l * (l + 1) // 2 + (k - 1)


def build_plan(cfg, k_edge_index, batch):
    NC, L, W = cfg.NCORES, cfg.L, cfg.W
    kei = np.asarray(k_edge_index)
    hops = []
    for k in range(1, L + 1):
        src = np.asarray(kei[0, (k - 1) * cfg.E_K: k * cfg.E_K], np.int64)
        dst = np.asarray(kei[1, (k - 1) * cfg.E_K: k * cfg.E_K], np.int64)
        deg = np.bincount(dst, minlength=cfg.N).astype(np.float64) + 1.0
        dinv = 1.0 / np.sqrt(deg)
        per_core = []
        T = np.zeros((W, 2), np.int64)
        for c in range(NC):
            lo, hi = cfg.core_lo(c), cfg.core_hi(c)
            m = (dst >= lo) & (dst < hi)
            es, ed = src[m], dst[m]
            erow = cfg.row_of(es)
            half = (erow & 1).astype(np.int64)
            pair = (erow >> 1).astype(np.int64)
            dl = ed - lo
            w = dl >> 7
            dr = dl & 127
            coefe = (dinv[es] * dinv[ed] / k).astype(np.float32)
            per_core.append((pair, half, w, dr, coefe, es))
            for h in (0, 1):
                cnt = np.bincount(w[half == h], minlength=W)
                T[:, h] = np.maximum(T[:, h], -(-cnt // 128))
        NT = [int(T[:, 0].sum()), int(T[:, 1].sum())]
        P = NT[0] + NT[1] + W
        # per-window bases
        TOFF = np.zeros((W, 2), np.int64)       # first stream-tile of window
        TOFF[1:, 0] = np.cumsum(T[:-1, 0])
        TOFF[1:, 1] = np.cumsum(T[:-1, 1])
        PB = np.zeros(W, np.int64)              # first piece of window
        PB[1:] = np.cumsum(T[:-1, 0] + T[:-1, 1] + 1)
        # piece index of each stream tile
        piece_of_tile = [np.zeros(max(NT[h], 1), np.int64) for h in (0, 1)]
        for w in range(W):
            for h in (0, 1):
                t0 = TOFF[w, h]
                pb = PB[w] + (T[w, 0] if h == 1 else 0)
                piece_of_tile[h][t0:t0 + T[w, h]] = pb + np.arange(T[w, h])
        idx_all = np.zeros((NC, 2, max(NT[0], 1) * 128), np.int16)
        dsr_all = np.full((NC, 128, P), 255.0, np.float32)
        coef_all = np.zeros((NC, 128, P), np.float32)
        if NT[1] != NT[0]:
            idx_all = np.zeros((NC, 2, max(NT[0], NT[1]) * 128), np.int16)
        src_at_pos = np.full((NC, 2, idx_all.shape[2]), -1, np.int64)
        for c in range(NC):
            pair, half, w, dr, coefe, esrc = per_core[c]
            lo, hi = cfg.core_lo(c), cfg.core_hi(c)
            for h in (0, 1):
                sel = half == h
                ww, pp, drr, cc = w[sel], pair[sel], dr[sel], coefe[sel]
                ee = esrc[sel]
                order = np.argsort(ww, kind='stable')
                ww, pp, drr, cc = ww[order], pp[order], drr[order], cc[order]
                ee = ee[order]
                cnts = np.bincount(ww, minlength=W)
                grp0 = np.concatenate([[0], np.cumsum(cnts)[:-1]])
                pos = TOFF[ww, h] * 128 + (np.arange(len(ww)) -
                                           np.repeat(grp0, cnts))
                idx_all[c, h, pos] = pp.astype(np.int16)
                src_at_pos[c, h, pos] = ee
                tg = pos >> 7
                part = pos & 127
                pc = piece_of_tile[h][tg]
                dsr_all[c, part, pc] = drr
                coef_all[c, part, pc] = cc
            # self pieces
            for w_ in range(W):
                pc = PB[w_] + T[w_, 0] + T[w_, 1]
                node = lo + w_ * 128 + np.arange(128)
                real = node < hi
                dsr_all[c, :, pc] = np.arange(128)
                cs = np.zeros(128, np.float32)
                cs[real] = (dinv[node[real]] ** 2 / k).astype(np.float32)
                coef_all[c, :, pc] = cs
                dsr_all[c, ~real, pc] = 255.0
        hops.append(dict(T=T, NT=NT, P=P, TOFF=TOFF, PB=PB,
                         idx=idx_all, dsr=dsr_all, coef=coef_all,
                         src=src_at_pos))
    # pooling slice->graph matrix
    Pm = np.zeros((NC, cfg.SMAX, cfg.GPAD), np.float32)
    for c in range(NC):
        lo, hi = cfg.core_lo(c), cfg.core_hi(c)
        gbase = lo // cfg.NPG
        for s in range((hi - lo) // cfg.NPG):
            Pm[c, s, gbase + s] = 1.0
    b = np.asarray(batch, np.int64)
    cnt = np.bincount(b, minlength=cfg.G)
    assert (cnt == cfg.NPG).all() and (np.sort(b) == b).all()
    key = tuple((h['NT'][0], h['NT'][1], h['P']) for h in hops)
    return dict(hops=hops, P=Pm, key=key)


def build_bass(cfg, plan):
    import concourse.bacc as bacc
    import concourse.mybir as mybir
    from concourse.tile import TileContext
    from concourse.library_config import mlp as mlp_lib

    f32, bf16, i16 = mybir.dt.float32, mybir.dt.bfloat16, mybir.dt.int16
    Alu = mybir.AluOpType
    Act = mybir.ActivationFunctionType
    AX = mybir.AxisListType.X
    NC, L, W = cfg.NCORES, cfg.L, cfg.W
    HID, GPAD, QCH = cfg.HID, cfg.GPAD, cfg.QCH
    RG = [list(range(NC))]

    nc = bacc.Bacc("TRN2", num_devices=NC)

    xT = nc.dram_tensor("xT", [cfg.IN_DIM, cfg.NLOC], f32, kind="ExternalInput")
    host_hops = {k for k, _ in HOST_PASSES}
    dev_hops = {op[1] for op in PROG if op[0] == 'pass' and op[3] != 'host'}
    idx_d, q_d, hostG_d = {}, {}, {}
    for k in range(1, L + 1):
        hp = plan['hops'][k - 1]
        ntmax = max(hp['NT'][0], hp['NT'][1], 1)
        if k in dev_hops:
            idx_d[k] = nc.dram_tensor(f"idx{k}", [2, 128, ntmax * 8], i16,
                                      kind="ExternalInput")
        q_d[k] = nc.dram_tensor(f"q{k}", [128, hp['P'] * 128], bf16,
                                kind="ExternalInput")
        if k in host_hops:
            hostG_d[k] = nc.dram_tensor(
                f"hg{k}", [128, (hp['NT'][0] + hp['NT'][1]) * 128], bf16,
                kind="ExternalInput")
    ident_d = nc.dram_tensor("ident", [128, 128], f32, kind="ExternalInput")
    P_d = nc.dram_tensor("P", [cfg.SMAX, GPAD], f32, kind="ExternalInput")
    embWT_d = nc.dram_tensor("embWT", [cfg.IN_DIM, HID], f32, kind="ExternalInput")
    embB_d = nc.dram_tensor("embB", [1, HID], f32, kind="ExternalInput")
    convWT_d = nc.dram_tensor("convWT", [cfg.NCONV, HID, HID], bf16,
                              kind="ExternalInput")
    biasL_d = nc.dram_tensor("biasL", [1, L * HID], bf16, kind="ExternalInput")
    r1WT_d = nc.dram_tensor("r1WT", [3 * HID, 192], f32, kind="ExternalInput")
    r1B_d = nc.dram_tensor("r1B", [192, 1], f32, kind="ExternalInput")
    r2WT_d = nc.dram_tensor("r2WT", [192, cfg.OUT], f32, kind="ExternalInput")
    r2B_d = nc.dram_tensor("r2B", [cfg.OUT, 1], f32, kind="ExternalInput")
    y_d = nc.dram_tensor("y", [cfg.OUT, GPAD], f32, kind="ExternalOutput")

    tmp = [nc.dram_tensor(f"tmp{j}", [cfg.TBL, HID], bf16, kind="Internal",
                          addr_space="Shared") for j in range(L)]
    wide = nc.dram_tensor("wide", [cfg.PAIRS, 2 * cfg.NSTRIPE * HID], bf16,
                          kind="Internal")
    hin = [nc.dram_tensor(f"hin{j}", [cfg.NLOC, HID], bf16, kind="Internal")
           for j in range(L)]
    ps_in = [nc.dram_tensor(f"pool_in{i}", [128, GPAD], f32, kind="Internal")
             for i in range(2)]
    ps_out = [nc.dram_tensor(f"pool_out{i}", [128, GPAD], f32, kind="Internal",
                             addr_space="Shared") for i in range(2)]

    # coverage check: every layer's xk gets seeded and every conv lands once
    seen = set()
    for op in PROG:
        if op[0] != 'pass':
            continue
        k, s0, nst = op[1], op[2], op[5]
        for jj in range(nst):
            seen.add(s0 + jj + k - 1)
    assert seen == {0, 1, 2, 3, 4}

    with TileContext(nc) as tc:
        nc.gpsimd.load_library(mlp_lib)
        with tc.tile_pool(name="const", bufs=1) as constp, \
             tc.tile_pool(name="persist", bufs=1) as pers, \
             tc.tile_pool(name="io", bufs=2) as iop, \
             tc.tile_pool(name="gp", bufs=2) as gp, \
             tc.tile_pool(name="qp", bufs=3) as qp, \
             tc.tile_pool(name="atp", bufs=3) as atp, \
             tc.tile_pool(name="selfp", bufs=3) as selfp, \
             tc.tile_pool(name="agg", bufs=4, space="PSUM") as aggp, \
             tc.tile_pool(name="outp", bufs=2, space="PSUM") as outp, \
             tc.tile_pool(name="smallps", bufs=2, space="PSUM") as smallp:

            ident = constp.tile([128, 128], f32)
            nc.sync.dma_start(ident[:], ident_d[:])
            ones_row = constp.tile([1, 128], f32)
            nc.vector.memset(ones_row[:], 1.0)
            ones_bf = constp.tile([1, 128], bf16)
            nc.vector.memset(ones_bf[:], 1.0)

            xkbuf = [pers.tile([128, W, HID], f32, tag=f"xk{i}",
                               name=f"xk{i}") for i in range(3)]
            h5T = xkbuf[0]      # dead by phase E; reuse as transpose staging

            blrow = constp.tile([1, L * HID], bf16)
            nc.sync.dma_start(blrow[:], biasL_d[:])
            biasbc = []
            for l in range(L):
                bps = smallp.tile([128, 128], f32, tag="smallt",
                                  name=f"bps{l}")
                nc.tensor.matmul(bps[:], ones_bf[:],
                                 blrow[0:1, l * HID:(l + 1) * HID])
                bb = constp.tile([128, 128], f32, name=f"biasbc{l}")
                nc.vector.tensor_copy(bb[:], bps[:])
                biasbc.append(bb)

            def emit_xkinit(l):
                nc.vector.tensor_copy(
                    xkbuf[XKMAP[l]][:, :, :],
                    biasbc[l][:, None, :].broadcast_to([128, W, HID]))

            # ---- Phase A: h0 = x @ embW^T + emb_b -> hin0 -> AG tmp0
            embWT = constp.tile([cfg.IN_DIM, HID], f32)
            nc.sync.dma_start(embWT[:], embWT_d[:])
            embB = constp.tile([1, HID], f32)
            nc.sync.dma_start(embB[:], embB_d[:])
            bias_ps = smallp.tile([128, 128], f32, tag="smallt")
            nc.tensor.matmul(bias_ps[:], ones_row[:], embB[:])
            embB_bc = constp.tile([128, 128], f32)
            nc.vector.tensor_copy(embB_bc[:], bias_ps[:])
            h0bf = pers.tile([128, W, HID], bf16, tag="hstage")
            for w in range(W):
                xtw = iop.tile([cfg.IN_DIM, 128], f32, tag="xtw")
                nc.sync.dma_start(xtw[:], xT[:, w * 128:(w + 1) * 128])
                hps = outp.tile([128, HID], f32, tag="ops")
                nc.tensor.matmul(hps[:], xtw[:], embWT[:])
                nc.vector.tensor_tensor(h0bf[:, w, :], hps[:], embB_bc[:], Alu.add)
            hview0 = hin[0].rearrange("(w j) f -> j w f", j=128)
            nc.sync.dma_start(hview0[:, :, :], h0bf[:])
            nc.gpsimd.collective_compute("AllGather", Alu.bypass,
                                         replica_groups=RG,
                                         ins=[hin[0][:]], outs=[tmp[0][:]])
            # interleave stripe 0 into wide
            wide_v = wide.rearrange("r (h s f) -> r h s f", h=2, s=cfg.NSTRIPE)
            tmp0_v = tmp[0].rearrange("(r two) f -> r two f", two=2)
            nc.sync.dma_start(wide_v[:, :, 0, :], tmp0_v[:])

            def emit_ilv(j):
                tv = tmp[j].rearrange("(r two) f -> r two f", two=2)
                nc.sync.dma_start(wide_v[:, :, j, :], tv[:])

            def emit_pass(k, s0, srck, tj, nst, wr=None):
                hp = plan['hops'][k - 1]
                T, NT, P, TOFF, PB = (hp['T'], hp['NT'], hp['P'],
                                      hp['TOFF'], hp['PB'])
                w_lo, w_hi = (0, W) if wr is None else wr
                if w_hi is None:
                    w_hi = W
                tbase = [int(TOFF[w_lo, 0]), int(TOFF[w_lo, 1])]
                tend = [int(TOFF[w_hi, h]) if w_hi < W else int(NT[h])
                        for h in (0, 1)]
                CH = CH_OF_NST[nst]
                uses = [(s0 + jj + k - 1, jj) for jj in range(nst)]
                wts = []
                for jj, (lp, _) in enumerate(uses):
                    wt = iop.tile([128, HID], bf16, tag=f"wt{jj}",
                                  name=f"wt{jj}")
                    nc.sync.dma_start(wt[:], convWT_d[ci_of(lp, k), :, :])
                    wts.append(wt)
                # gather source APs
                if srck == 'tmp':
                    tv = tmp[tj].rearrange("(r two) f -> r (two f)", two=2)
                    srcs = [tv[:, 0:HID], tv[:, HID:2 * HID]]
                    estep = 2 * HID
                elif srck == 'wide':
                    hw = cfg.NSTRIPE * HID
                    srcs = [wide[:, s0 * HID:(s0 + nst) * HID],
                            wide[:, hw + s0 * HID:hw + (s0 + nst) * HID]]
                    estep = 2 * hw

                if srck != 'host':
                    idx_sb = []
                    for h in (0, 1):
                        nth = max(NT[h], 1)
                        it = iop.tile([128, nth * 8], i16, tag=f"islab{h}",
                                      name=f"islab{h}")
                        nc.sync.dma_start(it[:], idx_d[k][h, :, 0:nth * 8])
                        idx_sb.append(it)

                gstate = [dict(c=-1, buf=None), dict(c=-1, buf=None)]

                def gtile(h, t):
                    st = gstate[h]
                    c = (t - tbase[h]) // CH
                    if c != st['c']:
                        t0 = tbase[h] + c * CH
                        n_t = min(CH, tend[h] - t0)
                        buf = gp.tile([128, CH, nst * HID], bf16, tag=f"g{h}")
                        if srck == 'host':
                            hoff = 0 if h == 0 else NT[0]
                            nc.sync.dma_start(
                                buf[:, 0:n_t, :].rearrange("p t f -> p (t f)"),
                                hostG_d[k][:, (hoff + t0) * HID:
                                           (hoff + t0 + n_t) * HID])
                        else:
                            nc.gpsimd.dma_gather(buf[:, 0:n_t, :], srcs[h],
                                                 idx_sb[h][:, t0 * 8:
                                                           (t0 + n_t) * 8],
                                                 n_t * 128, n_t * 128,
                                                 nst * HID, elem_step=estep,
                                                 single_packet=False)
                        st['c'], st['buf'], st['t0'] = c, buf, t0
                    return st['buf'][:, t - st['t0'], :]

                qstate = dict(c=-1, qv=None)

                def qpiece(p):
                    st = qstate
                    c = p // QCH
                    if c != st['c']:
                        n_p = min(QCH, P - c * QCH)
                        qv = qp.tile([128, QCH, 128], bf16, tag="qv", bufs=6)
                        nc.sync.dma_start(
                            qv[:, 0:n_p, :].rearrange("p t f -> p (t f)"),
                            q_d[k][:, c * QCH * 128:(c * QCH + n_p) * 128])
                        st['c'], st['qv'] = c, qv
                    return st['qv'][:, p - st['c'] * QCH, :]

                for w in range(w_lo, w_hi):
                    npieces = int(T[w, 0] + T[w, 1] + 1)
                    selfbs = []
                    for jj in range(nst):
                        sb = selfp.tile([128, HID], bf16, tag=f"sf{jj}")
                        nc.scalar.dma_start(
                            sb[:], hin[s0 + jj][w * 128:(w + 1) * 128, :])
                        selfbs.append(sb)
                    aggs = [aggp.tile([128, 128], f32, tag="agg",
                                      name=f"agg{jj}") for jj in range(nst)]
                    for pi in range(npieces):
                        q = qpiece(int(PB[w]) + pi)
                        first, last = pi == 0, pi == npieces - 1
                        if pi < T[w, 0]:
                            base = gtile(0, int(TOFF[w, 0]) + pi)
                            lhs = [base[:, jj * HID:(jj + 1) * HID]
                                   for jj in range(nst)]
                        elif pi < T[w, 0] + T[w, 1]:
                            t1 = pi - int(T[w, 0])
                            base = gtile(1, int(TOFF[w, 1]) + t1)
                            lhs = [base[:, jj * HID:(jj + 1) * HID]
                                   for jj in range(nst)]
                        else:
                            lhs = [selfbs[jj][:] for jj in range(nst)]
                        for jj in range(nst):
                            nc.tensor.matmul(aggs[jj][:], lhs[jj], q,
                                             start=first, stop=last)
                    for jj, (lp, _) in enumerate(uses):
                        at = atp.tile([128, 128], bf16, tag="at")
                        nc.scalar.copy(at[:], aggs[jj][:])
                        ops = outp.tile([128, 128], f32, tag="ops")
                        nc.tensor.matmul(ops[:], at[:], wts[jj][:],
                                         start=True, stop=True)
                        xkw = xkbuf[XKMAP[lp]][:, w, :]
                        nc.vector.tensor_tensor(xkw, xkw, ops[:], Alu.add)

            def emit_agstage(l):
                hbf = pers.tile([128, W, HID], bf16, tag="hstage")
                nc.scalar.activation(hbf[:], xkbuf[XKMAP[l]][:], Act.Relu)
                hv = hin[l + 1].rearrange("(w j) f -> j w f", j=128)
                nc.sync.dma_start(hv[:, :, :], hbf[:])
                nc.gpsimd.collective_compute(
                    "AllGather", Alu.bypass, replica_groups=RG,
                    ins=[hin[l + 1][:]], outs=[tmp[l + 1][:]])

            for op in PROG:
                if op[0] == 'pass':
                    _, k, s0, srck, tj, nst, *rest = op
                    emit_pass(k, s0, srck, tj, nst,
                              rest[0] if rest else None)
                elif op[0] == 'AGstage':
                    emit_agstage(op[1])
                elif op[0] == 'ilv':
                    emit_ilv(op[1])
                elif op[0] == 'xkinit':
                    emit_xkinit(op[1])

            # ---- final relu + pooling + MLP (per-window so Tile overlaps
            # the tail with the last gather pass)
            xk = xkbuf[XKMAP[L - 1]]
            for w in range(W):
                nc.scalar.activation(xk[:, w, :], xk[:, w, :], Act.Relu)
                tp = outp.tile([128, 128], f32, tag="ops")
                nc.tensor.transpose(tp[:], xk[:, w, :], ident[:])
                nc.vector.tensor_copy(h5T[:, w, :], tp[:])
            h5flat = h5T[:].rearrange("p w j -> p (w j)")
            ssum_l = iop.tile([128, cfg.SMAX], f32, tag="ssum_l", bufs=1)
            smax_l = iop.tile([128, cfg.SMAX], f32, tag="smax_l", bufs=1)
            nc.vector.memset(ssum_l[:], 0.0)
            nc.vector.memset(smax_l[:], 0.0)
            for s in range(cfg.SMAXR):
                sl = h5flat[:, s * cfg.NPG:(s + 1) * cfg.NPG]
                nc.vector.tensor_reduce(ssum_l[:, s:s + 1], sl, axis=AX,
                                        op=Alu.add)
                nc.vector.tensor_reduce(smax_l[:, s:s + 1], sl, axis=AX,
                                        op=Alu.max)
            Ps = constp.tile([cfg.SMAX, GPAD], f32)
            nc.sync.dma_start(Ps[:], P_d[:])
            ssum_t = iop.tile([cfg.SMAX, 128], f32, tag="ssum_t", bufs=1)
            smax_t = iop.tile([cfg.SMAX, 128], f32, tag="smax_t", bufs=1)
            for tsb, tsl in ((ssum_t, ssum_l), (smax_t, smax_l)):
                tps2 = smallp.tile([cfg.SMAX, 128], f32, tag="smallt",
                                   name="tps2")
                nc.tensor.transpose(tps2[:], tsl[:], ident[:])
                nc.vector.tensor_copy(tsb[:], tps2[:])
            for i, tsrc in enumerate((ssum_t, smax_t)):
                pps = outp.tile([128, GPAD], f32, tag="ops")
                nc.tensor.matmul(pps[:], tsrc[:], Ps[:])
                psb = iop.tile([128, GPAD], f32, tag=f"psb{i}", bufs=1)
                nc.vector.tensor_copy(psb[:], pps[:])
                nc.sync.dma_start(ps_in[i][:], psb[:])
            nc.gpsimd.collective_compute("AllReduce", Alu.add, replica_groups=RG,
                                         ins=[ps_in[0][:]], outs=[ps_out[0][:]])
            nc.gpsimd.collective_compute("AllReduce", Alu.max, replica_groups=RG,
                                         ins=[ps_in[1][:]], outs=[ps_out[1][:]])
            ssumR = pers.tile([128, GPAD], f32)
            nc.sync.dma_start(ssumR[:], ps_out[0][:])
            smaxR = pers.tile([128, GPAD], f32)
            nc.sync.dma_start(smaxR[:], ps_out[1][:])

            r1WTs = [constp.tile([HID, 192], f32, name=f"r1w{j}")
                     for j in range(3)]
            for j in range(3):
                nc.sync.dma_start(r1WTs[j][:], r1WT_d[j * HID:(j + 1) * HID, :])
            r1Bs = [constp.tile([128, 1], f32, name="r1b0"),
                    constp.tile([64, 1], f32, name="r1b1")]
            nc.sync.dma_start(r1Bs[0][:], r1B_d[0:128, :])
            nc.sync.dma_start(r1Bs[1][:], r1B_d[128:192, :])
            r2WTs = [constp.tile([128, cfg.OUT], f32, name="r2w0"),
                     constp.tile([64, cfg.OUT], f32, name="r2w1")]
            nc.sync.dma_start(r2WTs[0][:], r2WT_d[0:128, :])
            nc.sync.dma_start(r2WTs[1][:], r2WT_d[128:192, :])
            r2Bs = constp.tile([cfg.OUT, 1], f32)
            nc.sync.dma_start(r2Bs[:], r2B_d[:])
            chunks = (ssumR, smaxR, ssumR)   # mean folded into r1WT scale
            hidT = []
            for mi, (m0, m1) in enumerate(((0, 128), (128, 192))):
                hps2 = outp.tile([m1 - m0, GPAD], f32, tag="ops", name="hps2")
                for j in range(3):
                    nc.tensor.matmul(hps2[:], r1WTs[j][:, m0:m1],
                                     chunks[j][:], start=(j == 0), stop=(j == 2))
                hsb = iop.tile([m1 - m0, GPAD], f32, tag=f"hsb{m0}", bufs=1,
                               name=f"hsb{m0}")
                nc.scalar.activation(hsb[:], hps2[:], Act.Lrelu,
                                     bias=r1Bs[mi][:, 0:1], alpha=0.01)
                hidT.append(hsb)
            yps = outp.tile([cfg.OUT, GPAD], f32, tag="ops")
            nc.tensor.matmul(yps[:], r2WTs[0][:], hidT[0][:],
                             start=True, stop=False)
            nc.tensor.matmul(yps[:], r2WTs[1][:], hidT[1][:],
                             start=False, stop=True)
            ysb = iop.tile([cfg.OUT, GPAD], f32, tag="ysb", bufs=1)
            nc.scalar.activation(ysb[:], yps[:], Act.Identity,
                                 bias=r2Bs[:, 0:1])
            nc.sync.dma_start(y_d[:], ysb[:])

    nc.compile()
    return nc


def _wrap_idx(arr):
    w16 = arr.reshape(-1, 16).T
    return np.tile(w16, (8, 1)).copy()


def make_inmaps(cfg, plan, inputs):
    import ml_dtypes
    bf = ml_dtypes.bfloat16
    x = np.asarray(inputs['x'], np.float32)
    emb_W = np.asarray(inputs['emb_W'], np.float32)
    emb_b = np.asarray(inputs['emb_b'], np.float32)
    x0bf = (x @ emb_W.T + emb_b).astype(bf)
    host_hops = {k for k, _ in HOST_PASSES}
    conv_b = np.asarray(inputs['conv_b'], np.float32)
    biasL = np.zeros((cfg.L, cfg.HID), np.float32)
    for l in range(cfg.L):
        for k in range(1, l + 2):
            biasL[l] += conv_b[ci_of(l, k)] / k
    shared = dict(
        ident=np.eye(128, dtype=np.float32),
        embWT=np.ascontiguousarray(np.asarray(inputs['emb_W'], np.float32).T),
        embB=np.asarray(inputs['emb_b'], np.float32)[None, :].copy(),
        convWT=np.ascontiguousarray(
            np.asarray(inputs['conv_W'], np.float32).transpose(0, 2, 1)
        ).astype(bf),
        biasL=biasL.astype(bf).reshape(1, -1),
        r1WT=np.ascontiguousarray(
            np.asarray(inputs['r1_W'], np.float32).T
            * np.concatenate([np.ones(256, np.float32),
                              np.full(128, 0.01, np.float32)])[:, None]),
        r1B=np.asarray(inputs['r1_b'], np.float32)[:, None].copy(),
        r2WT=np.ascontiguousarray(np.asarray(inputs['r2_W'], np.float32).T),
        r2B=np.asarray(inputs['r2_b'], np.float32)[:, None].copy(),
    )
    in_maps = []
    for c in range(cfg.NCORES):
        m = dict(shared)
        lo, hi = cfg.core_lo(c), cfg.core_hi(c)
        xs = np.zeros((cfg.NLOC, cfg.IN_DIM), np.float32)
        xs[:hi - lo] = x[lo:hi]
        m['xT'] = np.ascontiguousarray(xs.T)
        for k in range(1, cfg.L + 1):
            hp = plan['hops'][k - 1]
            ntmax = max(hp['NT'][0], hp['NT'][1], 1)
            if k not in host_hops or k == 1:
                wrapped = np.zeros((2, 128, ntmax * 8), np.int16)
                for h in (0, 1):
                    nth = max(hp['NT'][h], 1)
                    wr = _wrap_idx(hp['idx'][c, h, :nth * 128])
                    wrapped[h, :, :nth * 8] = wr
                m[f'idx{k}'] = wrapped
            oneh = (np.arange(128, dtype=np.float32)[None, None, :] ==
                    hp['dsr'][c][:, :, None])
            qfull = (oneh * hp['coef'][c][:, :, None]).astype(bf)
            m[f'q{k}'] = np.ascontiguousarray(qfull.reshape(128, -1))
            if k in host_hops:
                NT0, NT1 = int(hp['NT'][0]), int(hp['NT'][1])
                full = np.zeros((128, NT0 + NT1, 128), bf)
                for h, base, nth in ((0, 0, NT0), (1, NT0, NT1)):
                    s = hp['src'][c, h, :nth * 128]
                    arr = x0bf[np.clip(s, 0, None)]
                    arr[s < 0] = 0
                    full[:, base:base + nth, :] = (
                        arr.reshape(nth, 128, 128).transpose(1, 0, 2))
                m[f'hg{k}'] = np.ascontiguousarray(full.reshape(128, -1))
        m['P'] = plan['P'][c]
        in_maps.append(m)
    return in_maps


_CACHE = {}


def kernel(**inputs):
    from concourse.bass_utils import run_bass_kernel_spmd
    cfg = Cfg()
    plan = build_plan(cfg, np.asarray(inputs['k_edge_index']),
                      np.asarray(inputs['batch']))
    if plan['key'] not in _CACHE:
        _CACHE[plan['key']] = build_bass(cfg, plan)
    nc = _CACHE[plan['key']]
    in_maps = make_inmaps(cfg, plan, inputs)
    res = run_bass_kernel_spmd(nc, in_maps, core_ids=list(range(cfg.NCORES)))
    out = np.asarray(res.results[0]['y'], np.float32)
    return np.ascontiguousarray(out[:, :cfg.G].T)

